# revision 39
# baseline (speedup 1.0000x reference)
"""Multi-head attention block (qkv -> attention -> o_net -> residual+LN) on
8 Trainium2 NeuronCores — head-parallel (tensor parallel) version.

Problem (hardcoded): B=2, T=2048, D=1024, H=16, dh=64, fp32 I/O.
Reference quirk: the (B,H,T,dh) attention buffer is viewed as (H,B,T,dh)
before the output projection: pair (b,h), g = 16*b + h, feeds OUTPUT batch
g % 2 through o_net column slot g // 2.  Heads 2c and 2c+1 share slot
8b + c: head 2c feeds OUT batch 0, head 2c+1 feeds OUT batch 1.

Sharding: core c owns heads {2c, 2c+1}.  Every core receives the FULL
input, computes q/k/v for its 2 heads over all 4096 tokens, runs attention
for its 4 (b,h) pairs, and applies its slice of o_net, producing partials
for BOTH out batches.  Partials are exchanged with AllToAll (NOT
ReduceScatter: RDH reduction channels cost ~200us setup per instance per
NEFF execution; AllToAll is pure DMA) and reduced locally.  Core c ends up
with out rows {out0[128c:+128], out0[1024+128c:+128], out1 same} —
residual + layernorm local.

Attention inner loop (per input batch b, per 128-query tile qt):
  scores   [keys, queries] via row-packed pairs (head A rows 0-63 of PE,
           head B rows 64-127, separate PSUM banks)
  exp      one ACT activation per 4-kt block ([128, 1024] = A|B halves)
  attn@V   flipped: lhsT = exp'd scores [128 keys, 128 q], rhs = V-block
           [128 keys, 130] (= headA 64 | ones 1 | headB 64 | ones 1) ->
           PSUM [128 q, 130]: both heads share one stream and the softmax
           denominator lands as a per-query COLUMN -> tensor_scalar
           normalize (no partition broadcast needed).
  transpose back to [head-dims, queries] (PE, host-provided identity);
  o_net as a row-packed pair: head-A half (out0) and head-B half (out1)
  concurrently into two PSUM banks.  b=0 partials persist in SBUF; b=1
  adds them during evacuation, then DMA to the AllToAll staging buffer.
"""
import sys
sys.path.insert(0, "/opt/trn_rl_repo")
import contextlib
import os as _os
import numpy as np
import ml_dtypes

import concourse.bass as bass
from concourse import bacc
import concourse.mybir as mybir
import concourse.tile as tile
from concourse.bass_utils import run_bass_kernel_spmd

BF16 = mybir.dt.bfloat16
F32 = mybir.dt.float32
nbf16 = ml_dtypes.bfloat16

N_CORES = 8
B, T, D = 2, 2048, 1024
H, DH = 16, 64
NT = B * T              # 4096 tokens
NQT = T // 128          # 16 query tiles per batch
LN_EPS = 1e-5
EXPF = mybir.ActivationFunctionType.Exp

_prog_cache = {}


def _build_program(reps=1):
    nc = bacc.Bacc("TRN2", num_devices=N_CORES)

    # ---- per-core inputs (host pre-tiled / pre-transposed, bf16) ----
    inpT = nc.dram_tensor("inpT", [128, 8, NT], BF16, kind="ExternalInput")
    inp_res = nc.dram_tensor("inp_res", [512, D], F32, kind="ExternalInput")
    wqkT = nc.dram_tensor("wqkT", [128, 8, 256], BF16, kind="ExternalInput")
    wvT = nc.dram_tensor("wvT", [128, 8, 128], BF16, kind="ExternalInput")
    woT = nc.dram_tensor("woT", [2, 128, D], BF16, kind="ExternalInput")
    b_qk = nc.dram_tensor("b_qk", [1, 256], BF16, kind="ExternalInput")
    b_v = nc.dram_tensor("b_v", [1, 128], BF16, kind="ExternalInput")
    onesd = nc.dram_tensor("onesd", [1, 512], BF16, kind="ExternalInput")
    identd = nc.dram_tensor("identd", [128, 128], BF16, kind="ExternalInput")
    gamma = nc.dram_tensor("gamma", [1, D], F32, kind="ExternalInput")
    beta = nc.dram_tensor("beta", [1, D], F32, kind="ExternalInput")

    out = nc.dram_tensor("out", [512, D], F32, kind="ExternalOutput")

    KPROBE = _os.environ.get("KPROBE", "")  # timing-only ablations
    KDBG = _os.environ.get("KDBG", "") == "1"
    if KDBG:
        dbg_k = nc.dram_tensor("dbg_k", [128, T], F32, kind="ExternalOutput")
        dbg_q = nc.dram_tensor("dbg_q", [128, T], F32, kind="ExternalOutput")
        dbg_v = nc.dram_tensor("dbg_v", [128, 130], F32, kind="ExternalOutput")
        dbg_po = nc.dram_tensor("dbg_po", [128, 130], F32, kind="ExternalOutput")
        dbg_nsb = nc.dram_tensor("dbg_nsb", [128, 128], F32, kind="ExternalOutput")

    def bcast_rows(src_row_ap, nrows):
        return bass.AP(tensor=src_row_ap.tensor, offset=src_row_ap.offset,
                       ap=[[0, nrows]] + src_row_ap.ap[1:])

    with tile.TileContext(nc) as tc:
        with contextlib.ExitStack() as ctx:
            dram = ctx.enter_context(tc.tile_pool(name="dram", bufs=1, space="DRAM"))
            cst = ctx.enter_context(tc.tile_pool(name="cst", bufs=1))

            # AllToAll staging: [half, 2048 rows, D].  Chunk h: shard r
            # (-> rank r) = rows [256r, 256r+256) = [out0 rows of qt 8h+r
            # (128) ; out1 rows of qt 8h+r (128)].
            a2a_in = dram.tile([2, 2048, D], BF16)
            a2a_out = dram.tile([2, 2048, D], BF16)

            ones_sb = cst.tile([1, 512], BF16)
            nc.sync.dma_start(out=ones_sb[:], in_=onesd[:])
            bqk_sb = cst.tile([1, 256], BF16)
            nc.sync.dma_start(out=bqk_sb[:], in_=b_qk[:])
            bv_sb = cst.tile([1, 128], BF16)
            nc.sync.dma_start(out=bv_sb[:], in_=b_v[:])
            ident_sb = cst.tile([128, 128], BF16)
            nc.sync.dma_start(out=ident_sb[:], in_=identd[:])
            wo_sb = cst.tile([128, 2, D], BF16)
            for b in range(2):
                nc.sync.dma_start(out=wo_sb[:, b, :], in_=woT[b, :, :])

            wqk_sb = cst.tile([128, 8, 256], BF16)
            nc.sync.dma_start(out=wqk_sb[:], in_=wqkT[:])
            wv_sb = cst.tile([128, 8, 128], BF16)
            nc.sync.dma_start(out=wv_sb[:], in_=wvT[:])

            # full input, transposed: [128, dt, token] (b0 tokens then b1)
            inpT_sb = cst.tile([128, 8, NT], BF16)
            for dt in range(8):
                nc.sync.dma_start(out=inpT_sb[:, dt, 0:T], in_=inpT[:, dt, 0:T])
            for dt in range(8):
                nc.sync.dma_start(out=inpT_sb[:, dt, T:NT], in_=inpT[:, dt, T:NT])

            # K^T/Q^T: [128 ch (headA 0:64 | headB 64:128), 2048 tokens]
            ksb = [cst.tile([128, T], BF16, name=f"ksb{b}") for b in range(2)]
            qsb = [cst.tile([128, T], BF16, name=f"qsb{b}") for b in range(2)]
            # V: [128 token-part, kt, 130] = headA 64 | ones | headB 64 | ones
            vsb = [cst.tile([128, NQT, 130], BF16, name=f"vsb{b}") for b in range(2)]

            # b=0 o_net partials, kept until b=1 adds them:
            # [128 q, qt%8, side, D]
            stq = cst.tile([128, 8, 2, D], BF16)

            gb_sb = cst.tile([128, D], F32)
            nc.gpsimd.dma_start(out=gb_sb[:], in_=bcast_rows(gamma[0:1, :], 128))
            bb_sb = cst.tile([128, D], F32)
            nc.gpsimd.dma_start(out=bb_sb[:], in_=bcast_rows(beta[0:1, :], 128))
            eps_sb = cst.tile([128, 1], F32)
            nc.vector.memset(eps_sb[:], LN_EPS)
            zrow = cst.tile([1, 130], BF16)
            nc.vector.memset(zrow[:], 0.0)

            # ---------------- qkv projection (both batches) ----------------
            def qkv(b, psproj):
                t0 = b * T
                nc.vector.memset(vsb[b][:, :, 64:65], 1.0)
                nc.vector.memset(vsb[b][:, :, 129:130], 1.0)
                for ch0, dst in ((128, ksb[b]), (0, qsb[b])):
                    for cc in range(4):
                        pp = psproj.tile([128, 512], F32, tag="pp")
                        nc.tensor.matmul(out=pp[:], lhsT=bqk_sb[0:1, ch0:ch0 + 128],
                                         rhs=ones_sb[:], start=True, stop=False)
                        for dt in range(8):
                            nc.tensor.matmul(
                                out=pp[:], lhsT=wqk_sb[:, dt, ch0:ch0 + 128],
                                rhs=inpT_sb[:, dt, t0 + cc * 512: t0 + (cc + 1) * 512],
                                start=False, stop=(dt == 7))
                        nc.vector.tensor_copy(out=dst[:, cc * 512:(cc + 1) * 512],
                                              in_=pp[:])
                for kt in range(NQT):
                    pv_full = psproj.tile([128, 512], F32, tag="pp")
                    pv = pv_full[:, 0:128]
                    nc.tensor.matmul(out=pv, lhsT=ones_sb[0:1, 0:128],
                                     rhs=bv_sb[:], start=True, stop=False)
                    for dt in range(8):
                        nc.tensor.matmul(
                            out=pv,
                            lhsT=inpT_sb[:, dt, t0 + kt * 128: t0 + (kt + 1) * 128],
                            rhs=wv_sb[:, dt, :], start=False, stop=(dt == 7))
                    nc.vector.tensor_copy(out=vsb[b][:, kt, 0:64], in_=pv[:, 0:64])
                    nc.vector.tensor_copy(out=vsb[b][:, kt, 65:129], in_=pv[:, 64:128])

            with tc.tile_pool(name="psproj", bufs=4, space="PSUM") as psproj:
                qkv(0, psproj)
                qkv(1, psproj)

            def dbg_dump(dst, src_ap, shape):
                t = cst.tile(list(shape), F32, name=f"dbgt_{dst.name}")
                nc.vector.tensor_copy(out=t[:], in_=src_ap)
                nc.sync.dma_start(out=dst[:, :], in_=t[:])

            if KDBG:
                dbg_dump(dbg_k, ksb[0][:, :], (128, T))
                dbg_dump(dbg_q, qsb[0][:, :], (128, T))
                dbg_dump(dbg_v, vsb[0][:, 0, :], (128, 130))

            # -------- attention + o_net + AllToAll + reduce (repeated) -----
            for _rep in range(reps):
              with tc.tile_pool(name="pss", bufs=2, space="PSUM") as pss, \
                 tc.tile_pool(name="pso", bufs=1, space="PSUM") as pso, \
                 tc.tile_pool(name="psT", bufs=1, space="PSUM") as psT, \
                 tc.tile_pool(name="pson", bufs=1, space="PSUM") as pson, \
                 tc.tile_pool(name="ptp", bufs=3) as ptp, \
                 tc.tile_pool(name="nrm", bufs=3) as nrm, \
                 tc.tile_pool(name="avp", bufs=2) as avp, \
                 tc.tile_pool(name="stp", bufs=3) as stp, \
                 tc.tile_pool(name="fin", bufs=1) as fin, \
                 tc.tile_pool(name="rsl", bufs=1) as rsl:

                def attn_part1(b, qt):
                    q0 = qt * 128
                    po = pso.tile([128, 130], F32, tag="po")
                    # start=True pending-zeroes the whole 2KB PSUM zero-region,
                    # so interleaved A/B groups in one bank would clobber each
                    # other: zero the full region once, then only accumulate.
                    nc.tensor.matmul(out=po[:], lhsT=zrow[0:1, 0:128],
                                     rhs=zrow[0:1, 0:130], start=True, stop=False,
                                     skip_group_check=True)
                    for blk in range(4):
                        pscr = pss.tile([128, 1024], F32, tag="pscr")
                        for j in range(4):
                            kt = blk * 4 + j
                            nc.tensor.matmul(
                                out=pscr[:, j * 128:(j + 1) * 128],
                                lhsT=ksb[b][0:64, kt * 128:(kt + 1) * 128],
                                rhs=qsb[b][0:64, q0:q0 + 128],
                                start=True, stop=True, tile_position=(0, 0))
                            nc.tensor.matmul(
                                out=pscr[:, 512 + j * 128: 512 + (j + 1) * 128],
                                lhsT=ksb[b][64:128, kt * 128:(kt + 1) * 128],
                                rhs=qsb[b][64:128, q0:q0 + 128],
                                start=True, stop=True, tile_position=(64, 0))
                        pt = ptp.tile([128, 1024], BF16, tag="pt")
                        nc.scalar.activation(out=pt[:], in_=pscr[:], func=EXPF,
                                             scale=0.125)
                        for j in range(4):
                            kt = blk * 4 + j
                            nc.tensor.matmul(
                                out=po[:, 0:65],
                                lhsT=pt[:, j * 128:(j + 1) * 128],
                                rhs=vsb[b][:, kt, 0:65],
                                start=False, stop=False,
                                skip_group_check=True)
                            nc.tensor.matmul(
                                out=po[:, 65:130],
                                lhsT=pt[:, 512 + j * 128: 512 + (j + 1) * 128],
                                rhs=vsb[b][:, kt, 65:130],
                                start=False, stop=(kt == 15),
                                skip_group_check=True)

                    recA = nrm.tile([128, 1], F32, tag="recA")
                    nc.vector.reciprocal(out=recA[:], in_=po[:, 64:65])
                    recB = nrm.tile([128, 1], F32, tag="recB")
                    nc.vector.reciprocal(out=recB[:], in_=po[:, 129:130])
                    nsb = nrm.tile([128, 128], BF16, tag="nsb")
                    nc.vector.tensor_scalar_mul(out=nsb[:, 0:64], in0=po[:, 0:64],
                                                scalar1=recA[:])
                    nc.vector.tensor_scalar_mul(out=nsb[:, 64:128], in0=po[:, 65:129],
                                                scalar1=recB[:])
                    if KDBG and b == 0 and qt == 0 and _rep == 0:
                        dbg_dump(dbg_po, po[:, :], (128, 130))
                        dbg_dump(dbg_nsb, nsb[:, :], (128, 128))
                    return nsb

                def attn_part2(b, qt, nsb):
                    # transpose to [head-dims, queries] for o_net
                    pt2 = psT.tile([128, 128], BF16, tag="pt2")
                    nc.tensor.transpose(out=pt2[0:64, :], in_=nsb[:, 0:64],
                                        identity=ident_sb[:])
                    nc.tensor.transpose(out=pt2[64:128, :], in_=nsb[:, 64:128],
                                        identity=ident_sb[:])
                    av = avp.tile([128, 128], BF16, tag="av")
                    nc.vector.tensor_copy(out=av[:], in_=pt2[:])

                    # o_net row-packed pair: head A (-> out0) on PE rows 0:63,
                    # head B (-> out1) on rows 64:127, different PSUM banks.
                    qm = qt % 8
                    if b == 1:
                        st = stp.tile([128, 2, D], BF16, tag="st")
                    for cc in range(2):
                        po2 = pson.tile([128, 1024], F32, tag="po2")
                        nc.tensor.matmul(
                            out=po2[:, 0:512], lhsT=av[0:64, :],
                            rhs=wo_sb[0:64, b, cc * 512:(cc + 1) * 512],
                            start=True, stop=True, tile_position=(0, 0))
                        nc.tensor.matmul(
                            out=po2[:, 512:1024], lhsT=av[64:128, :],
                            rhs=wo_sb[64:128, b, cc * 512:(cc + 1) * 512],
                            start=True, stop=True, tile_position=(64, 0))
                        # strided dst: [side 0 cols cc*512.. | side 1 cols ..]
                        po2v = po2[:].rearrange("p (s c) -> p s c", c=512)
                        if b == 0:
                            dst = bass.AP(
                                tensor=stq.tensor,
                                offset=stq.offset + qm * 2 * D + cc * 512,
                                ap=[stq[:].ap[0], [D, 2], [1, 512]])
                            nc.vector.tensor_copy(out=dst, in_=po2v)
                        else:
                            dst = bass.AP(
                                tensor=st.tensor, offset=st.offset + cc * 512,
                                ap=[st[:].ap[0], [D, 2], [1, 512]])
                            src1 = bass.AP(
                                tensor=stq.tensor,
                                offset=stq.offset + qm * 2 * D + cc * 512,
                                ap=[stq[:].ap[0], [D, 2], [1, 512]])
                            nc.vector.tensor_tensor(out=dst, in0=po2v, in1=src1,
                                                    op=mybir.AluOpType.add)
                    if b == 1:
                        half = qt // 8
                        row = 256 * qm
                        nc.sync.dma_start(out=a2a_in[half, row:row + 128, :],
                                          in_=st[:, 0, :])
                        nc.sync.dma_start(out=a2a_in[half, row + 128:row + 256, :],
                                          in_=st[:, 1, :])

                def a2a_issue(half):
                    nc.gpsimd.collective_compute(
                        "AllToAll", mybir.AluOpType.bypass,
                        replica_groups=[list(range(8))],
                        ins=[a2a_in[half].rearrange("r d -> (r d)")],
                        outs=[a2a_out[half].rearrange("r d -> (r d)")],
                    )

                def finalize(half):
                    for side in range(2):
                        # load this side's 8 received blocks
                        bl = []
                        for s in range(8):
                            t = rsl.tile([128, D], BF16, tag=f"rb{s}",
                                         name=f"rb{s}")
                            nc.sync.dma_start(
                                out=t[:],
                                in_=a2a_out[half, s * 256 + side * 128:
                                            s * 256 + side * 128 + 128, :])
                            bl.append(t)
                        # two accumulator chains: DVE takes 0-3, Pool 4-7
                        sD = fin.tile([128, D], F32, tag="sD")
                        nc.vector.tensor_tensor(out=sD[:], in0=bl[0][:], in1=bl[1][:],
                                                op=mybir.AluOpType.add)
                        sP = fin.tile([128, D], F32, tag="sP")
                        nc.gpsimd.tensor_tensor(out=sP[:], in0=bl[4][:], in1=bl[5][:],
                                                op=mybir.AluOpType.add)
                        for i in (2, 3):
                            nc.vector.tensor_tensor(out=sD[:], in0=sD[:],
                                                    in1=bl[i][:],
                                                    op=mybir.AluOpType.add)
                        for i in (6, 7):
                            nc.gpsimd.tensor_tensor(out=sP[:], in0=sP[:],
                                                    in1=bl[i][:],
                                                    op=mybir.AluOpType.add)
                        lrow = 128 * half + 256 * side
                        res_t = fin.tile([128, D], F32, tag="res_t")
                        nc.sync.dma_start(out=res_t[:],
                                          in_=inp_res[lrow:lrow + 128, :])
                        x = fin.tile([128, D], F32, tag="x")
                        nc.vector.tensor_tensor(out=x[:], in0=sD[:], in1=sP[:],
                                                op=mybir.AluOpType.add)
                        nc.vector.tensor_tensor(out=x[:], in0=x[:],
                                                in1=res_t[:],
                                                op=mybir.AluOpType.add)
                        stats = fin.tile([128, 2, 6], F32, tag="stats")
                        for s2 in range(2):
                            nc.vector.bn_stats(out=stats[:, s2, :],
                                               in_=x[:, s2 * 512:(s2 + 1) * 512])
                        mv = fin.tile([128, 2], F32, tag="mv")
                        nc.vector.bn_aggr(out=mv[:], in_=stats[:])
                        # rstd = exp(-0.5 * ln(var + eps)): Ln and Exp share
                        # one ACT table set, so no table swap after attention
                        lnv = fin.tile([128, 1], F32, tag="lnv")
                        nc.scalar.activation(out=lnv[:], in_=mv[:, 1:2],
                                             func=mybir.ActivationFunctionType.Ln,
                                             bias=eps_sb[:], scale=1.0)
                        rstd = fin.tile([128, 1], F32, tag="rstd")
                        nc.scalar.activation(out=rstd[:], in_=lnv[:],
                                             func=EXPF, scale=-0.5)
                        y = fin.tile([128, D], F32, tag="y")
                        nc.vector.tensor_scalar(out=y[:], in0=x[:],
                                                scalar1=mv[:, 0:1], scalar2=rstd[:],
                                                op0=mybir.AluOpType.subtract,
                                                op1=mybir.AluOpType.mult)
                        nc.gpsimd.tensor_tensor(out=y[:], in0=y[:], in1=gb_sb[:],
                                                op=mybir.AluOpType.mult)
                        nc.vector.tensor_tensor(out=y[:], in0=y[:], in1=bb_sb[:],
                                                op=mybir.AluOpType.add)
                        nc.sync.dma_start(out=out[lrow:lrow + 128, :], in_=y[:])

                # tile order: (b0 low-half, b1 low-half) -> A2A(0) ->
                # (b0 high, b1 high) -> A2A(1); 1-tile part1/part2 pipeline
                tiles = [(b, qt) for half in range(2) for b in range(2)
                         for qt in range(8 * half, 8 * half + 8)]
                pending = None
                for t in tiles:
                    b, qt = t
                    nsb = attn_part1(b, qt)
                    if pending is not None:
                        pb, pqt = pending[0]
                        attn_part2(pb, pqt, pending[1])
                        if pb == 1 and pqt % 8 == 7 and KPROBE != "nors":
                            a2a_issue(pqt // 8)
                    pending = (t, nsb)
                pb, pqt = pending[0]
                attn_part2(pb, pqt, pending[1])
                if KPROBE != "nors":
                    a2a_issue(1)
                    # finalize(0) is emitted only now: emitted any earlier,
                    # its DVE ops would block the in-order DVE queue waiting
                    # on A2A(0).  Here it runs concurrently with A2A(1).
                    finalize(0)
                    finalize(1)
                else:
                    if _rep == reps - 1:
                        z = fin.tile([128, D], F32, tag="x")
                        nc.vector.memset(z[:], 0.0)
                        for chunk in range(4):
                            nc.sync.dma_start(
                                out=out[chunk * 128:(chunk + 1) * 128, :], in_=z[:])

    nc.finalize()
    return nc


def _get_program(reps=1):
    key = (reps, _os.environ.get("KPROBE", ""), _os.environ.get("KDBG", ""))
    if key not in _prog_cache:
        _prog_cache[key] = _build_program(reps)
    return _prog_cache[key]


def _prep_inputs(inp, W_qkv, b_qkv, W_o, gamma, beta):
    """Build the 8 per-core input dicts (host-side, all free)."""
    f32 = np.float32
    inp = np.asarray(inp, f32)
    W_qkv = np.asarray(W_qkv, f32)
    b_qkv = np.asarray(b_qkv, f32)
    W_o = np.asarray(W_o, f32)
    gamma = np.asarray(gamma, f32).reshape(1, D)
    beta = np.asarray(beta, f32).reshape(1, D)

    x = np.concatenate([inp[0], inp[1]], axis=0)              # [4096, 1024]
    inpT = np.ascontiguousarray(
        x.T.reshape(8, 128, NT).transpose(1, 0, 2)).astype(nbf16)
    ones = np.ones((1, 512), nbf16)
    ident = np.eye(128, dtype=nbf16)

    in_maps = []
    for c in range(N_CORES):
        hA, hB = 2 * c, 2 * c + 1
        # qkv channel rows: [qA, qB, kA, kB] then [vA, vB]
        qk_rows = np.r_[64 * hA: 64 * hA + 64, 64 * hB: 64 * hB + 64,
                        1024 + 64 * hA: 1024 + 64 * hA + 64,
                        1024 + 64 * hB: 1024 + 64 * hB + 64]
        v_rows = np.r_[2048 + 64 * hA: 2048 + 64 * hA + 64,
                       2048 + 64 * hB: 2048 + 64 * hB + 64]
        wqkT = np.ascontiguousarray(
            W_qkv[qk_rows, :].T.reshape(8, 128, 256).transpose(1, 0, 2)).astype(nbf16)
        wvT = np.ascontiguousarray(
            W_qkv[v_rows, :].T.reshape(8, 128, 128).transpose(1, 0, 2)).astype(nbf16)
        bqk = b_qkv[qk_rows].reshape(1, 256).astype(nbf16)
        bv = b_qkv[v_rows].reshape(1, 128).astype(nbf16)
        # o_net: heads 2c (->out0) and 2c+1 (->out1) share slot 8b + c;
        # woT[b] rows 0:64 and 64:128 both hold Wo[:, 64*(8b+c):+64].T
        woT = np.empty((2, 128, D), nbf16)
        for b in range(2):
            s = 8 * b + c
            wslice = W_o[:, 64 * s: 64 * s + 64].T.astype(nbf16)
            woT[b, 0:64] = wslice
            woT[b, 64:128] = wslice
        # residual rows: out0[128c], out0[1024+128c], out1[128c], out1[1024+128c]
        res = np.concatenate([
            inp[0, 128 * c: 128 * c + 128, :],
            inp[0, 1024 + 128 * c: 1024 + 128 * c + 128, :],
            inp[1, 128 * c: 128 * c + 128, :],
            inp[1, 1024 + 128 * c: 1024 + 128 * c + 128, :],
        ], axis=0)
        in_maps.append({
            "inpT": inpT,
            "inp_res": np.ascontiguousarray(res),
            "wqkT": wqkT, "wvT": wvT, "woT": woT,
            "b_qk": bqk, "b_v": bv, "onesd": ones, "identd": ident,
            "gamma": gamma, "beta": beta,
        })
    return in_maps


def _assemble(results):
    out = np.empty((B, T, D), np.float32)
    for c in range(N_CORES):
        o = results[c]["out"]
        out[0, 128 * c: 128 * c + 128, :] = o[0:128]
        out[0, 1024 + 128 * c: 1024 + 128 * c + 128, :] = o[128:256]
        out[1, 128 * c: 128 * c + 128, :] = o[256:384]
        out[1, 1024 + 128 * c: 1024 + 128 * c + 128, :] = o[384:512]
    return out


def kernel(inp, W_qkv, b_qkv, W_o, gamma, beta):
    nc = _get_program()
    in_maps = _prep_inputs(inp, W_qkv, b_qkv, W_o, gamma, beta)
    res = run_bass_kernel_spmd(nc, in_maps, core_ids=list(range(N_CORES)))
    return _assemble(res.results)


if __name__ == "__main__":
    rng = np.random.RandomState(0)
    inp = rng.randn(B, T, D).astype(np.float32)
    W_qkv = (rng.randn(3 * H * DH, D) * D ** -0.5).astype(np.float32)
    b_qkv = (rng.randn(3 * H * DH) * 0.02).astype(np.float32)
    W_o = (rng.randn(D, H * DH) * (H * DH) ** -0.5).astype(np.float32)
    gamma = np.ones(D, np.float32)
    beta = np.zeros(D, np.float32)
    out = kernel(inp=inp, W_qkv=W_qkv, b_qkv=b_qkv, W_o=W_o, gamma=gamma, beta=beta)
    print("out", out.shape, out.dtype, np.abs(out).mean())
